# revision 9
# baseline (speedup 1.0000x reference)
"""Cross-attention (B=2, Q=1024, N=4096, C=768, H=12, D=64) with bilinearly
interpolated relative position bias, on 8 Trainium2 NeuronCores.

Sharding: core c handles batch b = c//4 and heads 3*(c%4) .. 3*(c%4)+2
(tensor-parallel over heads, data-parallel over batch). Each core outputs, per
head, the unnormalized attention output projected through Wo_h (c-major), plus
the softmax denominators; the host divides, sums the partials, and adds bo.

Device algorithm per core (fp16 matmul operands, fp32 accumulation):
  qbT[h]  = (Wq_h^T @ q^T) * scale + bq          [64, 1024]   (d-major)
  kbT[h]  = Wk_h^T @ kv^T + bk                   [64, 4096]
  vb[n]   = kv @ Wv_h + bv                       [4096, 64]   (n-major)
  S^T     = [kbT; Wn]^T-contraction [qbT; B1T]   K=96 fuses the interpolated
            bias: bias[h,q,n] = sum_j B1[h,q,j] * Wn[j,n]
  E^T     = exp(S^T)            (no max-subtraction; logits are O(1))
  O^T[h]  = [vb_h | 1]^T @ E^T                   [65, 1024]  row 64 = denom
  G^T[h]  = Wo_h^T-contraction O^T[h]            [768, 1024]  (c-major, unnorm)

Scheduling: kv is DMA'd in 8 column chunks ordered so the k/v projection of
chunk 0 (B0) can start ~2.5us in; a short warm-up matmul burst covers the
initial DMA latency and holds the HAM clock gate at 8/8.  The k/v projections
of chunks 2-7 are split into 4 sub-parts each and emitted one sub-part per
attention chunk of head 0 so the exp pipeline is never starved by a long
projection burst.  Wo for heads 0/1 is spread one (cs,qh) slice per attention
chunk of heads 1/2; head 2's Wo forms the tail with copies alternating between
the vector and scalar engines.
"""

import numpy as np

B, Q, N, C = 2, 1024, 4096, 768
H, D, REL = 12, 64, 32
SCALE = 1.0 / np.sqrt(D)
HPC = 3            # heads per core
N_CORES = 8
NCH = 8            # 512-wide n-chunks

_COMPILED = None   # cached nc across kernel() calls


def _lin_coords(n_out, n_in):
    pos = np.arange(n_out, dtype=np.float32) * np.float32((n_in - 1) / (n_out - 1))
    lo = np.clip(np.floor(pos).astype(np.int32), 0, n_in - 1)
    hi = np.clip(lo + 1, 0, n_in - 1)
    w = (pos - lo.astype(np.float32)).astype(np.float32)
    return lo, hi, w


def _host_bias_parts(rel_pos_bias):
    """B1: [H, Q, 32] q-interpolated bias; Wn: [32, N] n-interp weights."""
    lq, hq, wq = _lin_coords(Q, REL)
    ln, hn, wn = _lin_coords(N, REL)
    b1 = (rel_pos_bias[:, lq, :] * (1.0 - wq)[None, :, None]
          + rel_pos_bias[:, hq, :] * wq[None, :, None]).astype(np.float32)
    w_n = np.zeros((REL, N), np.float32)
    np.add.at(w_n, (ln, np.arange(N)), (1.0 - wn))
    np.add.at(w_n, (hn, np.arange(N)), wn)
    return b1, w_n


def _build():
    import concourse.tile as tile
    from concourse import bacc, mybir
    import concourse.bass as bass

    F32 = mybir.dt.float32
    F16 = mybir.dt.float16
    KT = 6  # C // 128 contraction tiles

    nc = bacc.Bacc("TRN2", target_bir_lowering=False, debug=False,
                   enable_asserts=False, num_devices=N_CORES)

    # DRAM layouts are partition-major / chunk-contiguous so every transfer
    # is one dma_start with large contiguous runs — the DGE descriptor
    # generator on the Sync queue is the startup bottleneck, not HBM BW.
    qT = nc.dram_tensor("qT", [128, KT, Q], F16, kind="ExternalInput")
    kvd = nc.dram_tensor("kvd", [NCH, 128, KT, 512], F16, kind="ExternalInput")
    wq = nc.dram_tensor("wq", [128, KT, 192], F16, kind="ExternalInput")  # pre-scaled
    wk = nc.dram_tensor("wk", [128, KT, 192], F16, kind="ExternalInput")
    wv = nc.dram_tensor("wv", [128, KT, 192], F16, kind="ExternalInput")
    wo = nc.dram_tensor("wo", [D, HPC, C], F16, kind="ExternalInput")
    bqs = nc.dram_tensor("bqs", [D, HPC], F32, kind="ExternalInput")   # pre-scaled
    bks = nc.dram_tensor("bks", [D, HPC], F32, kind="ExternalInput")
    bvb = nc.dram_tensor("bvb", [128, 192], F32, kind="ExternalInput")  # bcast bv
    b1t = nc.dram_tensor("b1t", [HPC, REL, Q], F16, kind="ExternalInput")
    w_n = nc.dram_tensor("w_n", [REL, N], F16, kind="ExternalInput")
    out_p = nc.dram_tensor("out_p", [HPC, 6, 128, 2, 512], F16,
                           kind="ExternalOutput")
    den = nc.dram_tensor("den", [HPC, Q], F16, kind="ExternalOutput")

    EXP = mybir.ActivationFunctionType.Exp
    ADD = mybir.AluOpType.add

    with tile.TileContext(nc) as tc:
        with (
            tc.tile_pool(name="wpool", bufs=1) as wpool,
            tc.tile_pool(name="persist", bufs=1) as pers,
            tc.tile_pool(name="psS", bufs=2, space="PSUM") as psSp,
            tc.tile_pool(name="pexp", bufs=4) as pexp,
            tc.tile_pool(name="tailp", bufs=2) as tailp,
        ):
            # ---- PE warm-up: keep the array busy until the first DMAs land
            # so the HAM clock gate reaches 8/8; also preloads the Exp table.
            scr = wpool.tile([128, 256], F16, name="scr")
            nc.vector.memset(scr, 0.0)
            dum = wpool.tile([1, 16], F32, name="dum")
            nc.vector.memset(dum, 0.0)
            dux = wpool.tile([1, 16], F16, name="dux")
            nc.scalar.activation(out=dux, in_=dum, func=EXP)
            with tc.tile_pool(name="pswrm", bufs=1, space="PSUM") as pswrm:
                psw = pswrm.tile([128, 256], F32, name="psw")
                for _ in range(24):
                    nc.tensor.matmul(psw, scr[:, 0:128], scr,
                                     start=True, stop=True)

            # ---- inputs: one dma_start per tensor (or per kv chunk), in
            # priority order.  The DGE issues them serially, so program
            # order IS the priority order. ----
            kv_sb = wpool.tile([128, NCH, KT, 512], F16, name="kv_sb")
            wk_sb = wpool.tile([128, KT, 192], F16, name="wk_sb")
            wv_sb = wpool.tile([128, KT, 192], F16, name="wv_sb")
            wq_sb = wpool.tile([128, KT, 192], F16, name="wq_sb")
            qT_sb = wpool.tile([128, KT, Q], F16, name="qT_sb")
            wo_sb = wpool.tile([D, HPC, C], F16, name="wo_sb")
            bks_sb = wpool.tile([D, HPC], F32, name="bks_sb")
            bvb_sb = wpool.tile([128, 192], F32, name="bvb_sb")
            bqs_sb = wpool.tile([D, HPC], F32, name="bqs_sb")
            wn_sb = wpool.tile([128, N], F16, name="wn_sb")  # rows 64:96 used
            # Two parallel DGE queues: phase A's inputs go through the
            # scalar engine's DGE (idle at startup), B0's through sync.
            nc.scalar.dma_start(out=wq_sb, in_=wq[:, :, :])
            nc.scalar.dma_start(out=bqs_sb, in_=bqs[:, :])
            nc.scalar.dma_start(out=qT_sb, in_=qT[:, :, :])
            nc.sync.dma_start(out=wk_sb, in_=wk[:, :, :])
            nc.sync.dma_start(out=wv_sb, in_=wv[:, :, :])
            nc.sync.dma_start(out=bks_sb, in_=bks[:, :])
            nc.sync.dma_start(out=bvb_sb, in_=bvb[:, :])
            nc.sync.dma_start(out=kv_sb[:, 0, :, :], in_=kvd[0, :, :, :])
            nc.sync.dma_start(out=wn_sb[64:96, :], in_=w_n[:, :])
            nc.sync.dma_start(out=kv_sb[:, 1, :, :], in_=kvd[1, :, :, :])

            # ---- persistent per-head / per-chunk tiles ----
            qTp = [pers.tile([96, Q], F16, name=f"qTp{h}", tag=f"qTp{h}")
                   for h in range(HPC)]
            for h in range(HPC):
                nc.scalar.dma_start(out=qTp[h][64:96, :], in_=b1t[h, :, :])
            kbTp = [[pers.tile([96, 512], F16, name=f"kbT{h}_{ch}",
                               tag=f"kbT{h}_{ch}")
                     for ch in range(NCH)] for h in range(HPC)]
            vb = [pers.tile([128, 195], F16, name=f"vb{s}", tag=f"vb{s}")
                  for s in range(N // 128)]

            # ---- phase B sub-parts: k/v projections for one 512-wide n-chunk
            # split into 4 pieces so they interleave with attention chunks ----
            def emit_b_sub(ch, part, psB):
                ksl = slice(512 * ch, 512 * ch + 512)
                if part == 0:
                    pskA = psB.tile([128, 512], F32, name="pskA", tag="psb")
                    for t in range(KT):
                        nc.tensor.matmul(pskA, wk_sb[:, t, 0:128],
                                         kv_sb[:, ch, t, :],
                                         start=(t == 0), stop=(t == KT - 1))
                    nc.vector.tensor_scalar_add(kbTp[0][ch][0:64, :],
                                                pskA[0:64, :], bks_sb[:, 0:1])
                    nc.vector.tensor_scalar_add(kbTp[1][ch][0:64, :],
                                                pskA[64:128, :], bks_sb[:, 1:2])
                elif part == 1:
                    pskB = psB.tile([64, 512], F32, name="pskB", tag="psb")
                    for t in range(KT):
                        nc.tensor.matmul(pskB, wk_sb[:, t, 128:192],
                                         kv_sb[:, ch, t, :],
                                         start=(t == 0), stop=(t == KT - 1))
                    nc.vector.tensor_scalar_add(kbTp[2][ch][0:64, :],
                                                pskB[0:64, :], bks_sb[:, 2:3])
                    for h in range(HPC):
                        nc.vector.tensor_copy(out=kbTp[h][ch][64:96, :],
                                              in_=wn_sb[64:96, ksl])
                else:
                    for s in (0, 1) if part == 2 else (2, 3):
                        n128 = 4 * ch + s
                        psv = psB.tile([128, 192], F32, name="psv", tag="psb")
                        for t in range(KT):
                            nc.tensor.matmul(
                                psv, kv_sb[:, ch, t, 128 * s:128 * s + 128],
                                wv_sb[:, t, :],
                                start=(t == 0), stop=(t == KT - 1))
                        vt = vb[n128]
                        vt3 = vt[:, 0:195].rearrange("p (h e) -> p h e", e=65)
                        nc.vector.tensor_tensor(
                            out=vt3[:, :, 0:64],
                            in0=psv.rearrange("p (h d) -> p h d", d=64),
                            in1=bvb_sb.rearrange("p (h d) -> p h d", d=64),
                            op=ADD)
                        nc.vector.memset(vt3[:, :, 64:65], 1.0)

            def emit_b_chunk(ch, psB):
                for part in range(4):
                    emit_b_sub(ch, part, psB)

            # ---- attention S matmuls: one chunk of one head ----
            def emit_s(h, ci):
                c512, s = ci // 4, ci % 4
                ssl = slice(128 * s, 128 * s + 128)
                psS = psSp.tile([128, Q], F32, name="psS", tag="psS")
                nc.tensor.matmul(psS[:, 0:512], kbTp[h][c512][:, ssl],
                                 qTp[h][:, 0:512], start=True, stop=True)
                nc.tensor.matmul(psS[:, 512:1024], kbTp[h][c512][:, ssl],
                                 qTp[h][:, 512:1024], start=True, stop=True)
                return psS

            # ---- Wo: one (cs, qh) slice => 1 matmul + 1 copy + 1 dma ----
            def emit_wo_slice(h, onT, idx, psF, scalar_copy=False):
                cs, qh = idx // 2, idx % 2
                csl = slice(128 * cs, 128 * cs + 128)
                qsl = slice(512 * qh, 512 * qh + 512)
                psf = psF.tile([128, 512], F32, name="psf", tag="psf")
                nc.tensor.matmul(psf, wo_sb[:, h, csl], onT[0:64, qsl],
                                 start=True, stop=True)
                osb = tailp.tile([128, 512], F16, name="osb", tag="osb", bufs=4)
                if scalar_copy:
                    nc.scalar.copy(out=osb, in_=psf)
                else:
                    nc.vector.tensor_copy(out=osb, in_=psf)
                nc.sync.dma_start(out=out_p[h, cs, :, qh, :], in_=osb)

            import contextlib
            psB_ctx = contextlib.ExitStack()
            psB_pool = psB_ctx.enter_context(
                tc.tile_pool(name="psB", bufs=2, space="PSUM"))

            # ---- phase A: q projection ----
            with tc.tile_pool(name="psA0", bufs=1, space="PSUM") as psA0:
                for qc in range(2):
                    psqA = psA0.tile([128, 512], F32, name="psqA", tag="psqA")
                    psqB = psA0.tile([64, 512], F32, name="psqB", tag="psqB")
                    for t in range(KT):
                        nc.tensor.matmul(psqA, wq_sb[:, t, 0:128],
                                         qT_sb[:, t, 512 * qc:512 * qc + 512],
                                         start=(t == 0), stop=(t == KT - 1))
                        nc.tensor.matmul(psqB, wq_sb[:, t, 128:192],
                                         qT_sb[:, t, 512 * qc:512 * qc + 512],
                                         start=(t == 0), stop=(t == KT - 1))
                    sl = slice(512 * qc, 512 * qc + 512)
                    nc.vector.tensor_scalar_add(qTp[0][0:64, sl], psqA[0:64, :],
                                                bqs_sb[:, 0:1])
                    nc.vector.tensor_scalar_add(qTp[1][0:64, sl], psqA[64:128, :],
                                                bqs_sb[:, 1:2])
                    nc.vector.tensor_scalar_add(qTp[2][0:64, sl], psqB[0:64, :],
                                                bqs_sb[:, 2:3])

            emit_b_chunk(0, psB_pool)
            emit_b_chunk(1, psB_pool)
            for ch in range(2, NCH):
                nc.sync.dma_start(out=kv_sb[:, ch, :, :], in_=kvd[ch, :, :, :])
            nc.sync.dma_start(out=wo_sb, in_=wo[:, :, :])

            NCI = N // 128

            def emit_head(h, psB, wo_work):
                """wo_work: None or (h_prev, onT_prev, psF)."""
                import contextlib
                with contextlib.ExitStack() as pstk:
                    psO = pstk.enter_context(
                        tc.tile_pool(name=f"psO{h}", bufs=1, space="PSUM"))
                    po = psO.tile([65, Q], F32, name=f"po{h}", tag="po")
                    psS_cur = emit_s(h, 0)
                    wo_idx = 0
                    for ci in range(NCI):
                        # run-ahead: next chunk's S matmuls first so exp
                        # never waits on them
                        if ci + 1 < NCI:
                            psS_nxt = emit_s(h, ci + 1)
                        # one sub-part of a pending k/v projection chunk
                        if psB is not None and ci < 24:
                            emit_b_sub(2 + ci // 4, ci % 4, psB)
                        # one Wo slice of the previous head
                        if wo_work is not None and 2 <= ci and wo_idx < 12:
                            hp, onTp, psF = wo_work
                            emit_wo_slice(hp, onTp, wo_idx, psF)
                            wo_idx += 1
                        px = pexp.tile([128, Q], F16, name="px", tag="px")
                        nc.scalar.activation(out=px, in_=psS_cur, func=EXP)
                        st = (ci == 0)
                        sp = (ci == NCI - 1)
                        nc.tensor.matmul(po[:, 0:512],
                                         vb[ci][:, 65 * h:65 * h + 65],
                                         px[:, 0:512], start=st, stop=sp)
                        nc.tensor.matmul(po[:, 512:1024],
                                         vb[ci][:, 65 * h:65 * h + 65],
                                         px[:, 512:1024], start=st, stop=sp)
                        if ci + 1 < NCI:
                            psS_cur = psS_nxt
                    # single copy grabs the 64 output dims AND the denom row
                    onT = tailp.tile([65, Q], F16, name=f"onT{h}", tag="onT")
                    nc.vector.tensor_copy(out=onT, in_=po)
                    nc.sync.dma_start(out=den[h, :], in_=onT[64:65, :])
                return onT

            onT0 = emit_head(0, psB_pool, None)
            psB_ctx.close()
            psF_ctx = contextlib.ExitStack()
            psF_pool = psF_ctx.enter_context(
                tc.tile_pool(name="psF", bufs=2, space="PSUM"))
            onT1 = emit_head(1, None, (0, onT0, psF_pool))
            onT2 = emit_head(2, None, (1, onT1, psF_pool))
            # tail: head 2's Wo, copies alternating vector/scalar engines
            for idx in range(12):
                emit_wo_slice(2, onT2, idx, psF_pool, scalar_copy=(idx % 2 == 1))
            psF_ctx.close()
    nc.compile()
    return nc


def _get_compiled():
    global _COMPILED
    if _COMPILED is None:
        _COMPILED = _build()
    return _COMPILED


def _make_in_maps(query, key_value, Wq, bq, Wk, bk, Wv, bv, Wo, rel_pos_bias):
    b1, w_n = _host_bias_parts(rel_pos_bias)
    scale = np.float32(SCALE)
    f16 = np.float16
    qTs = [np.ascontiguousarray(
        query[b].T.reshape(6, 128, Q).transpose(1, 0, 2)).astype(f16)
        for b in range(B)]
    kvds = [np.ascontiguousarray(
        key_value[b].T.reshape(6, 128, NCH, 512).transpose(2, 1, 0, 3)
        ).astype(f16) for b in range(B)]
    def wlay(w):
        return np.ascontiguousarray(
            w.reshape(6, 128, 192).transpose(1, 0, 2)).astype(f16)
    w_n16 = w_n.astype(f16)
    in_maps = []
    for c in range(N_CORES):
        b = c // (N_CORES // B)
        h0 = (c % (N_CORES // B)) * HPC
        cols = slice(D * h0, D * h0 + D * HPC)
        in_maps.append({
            "qT": qTs[b],
            "kvd": kvds[b],
            "wq": wlay(Wq[:, cols] * scale),
            "wk": wlay(Wk[:, cols]),
            "wv": wlay(Wv[:, cols]),
            "wo": np.ascontiguousarray(
                Wo[cols, :].reshape(HPC, D, C).transpose(1, 0, 2)).astype(f16),
            "bqs": np.ascontiguousarray((bq[cols] * scale).reshape(HPC, D).T),
            "bks": np.ascontiguousarray(bk[cols].reshape(HPC, D).T),
            "bvb": np.ascontiguousarray(
                np.broadcast_to(bv[cols][None, :], (128, D * HPC))),
            "b1t": np.ascontiguousarray(
                b1[h0:h0 + HPC].transpose(0, 2, 1)).astype(f16),
            "w_n": w_n16,
        })
    return in_maps


def kernel(query, key_value, Wq, bq, Wk, bk, Wv, bv, Wo, bo, rel_pos_bias):
    from concourse import bass_utils

    query = np.asarray(query, np.float32)
    key_value = np.asarray(key_value, np.float32)
    Wq = np.asarray(Wq, np.float32); bq = np.asarray(bq, np.float32)
    Wk = np.asarray(Wk, np.float32); bk = np.asarray(bk, np.float32)
    Wv = np.asarray(Wv, np.float32); bv = np.asarray(bv, np.float32)
    Wo = np.asarray(Wo, np.float32); bo = np.asarray(bo, np.float32)
    rel_pos_bias = np.asarray(rel_pos_bias, np.float32)

    in_maps = _make_in_maps(query, key_value, Wq, bq, Wk, bk, Wv, bv, Wo,
                            rel_pos_bias)
    nc = _get_compiled()
    res = bass_utils.run_bass_kernel_spmd(nc, in_maps,
                                          core_ids=list(range(N_CORES)))
    out = np.zeros((B, Q, C), np.float32)
    for c in range(N_CORES):
        b = c // (N_CORES // B)
        f = res.results[c]["out_p"].reshape(HPC, C, Q).astype(np.float32)
        d = res.results[c]["den"].astype(np.float32)     # [HPC, Q]
        out[b] += (f / d[:, None, :]).sum(axis=0).T
    out += bo[None, None, :]
    return out


# revision 10
# speedup vs baseline: 1.0287x; 1.0287x over previous
"""Cross-attention (B=2, Q=1024, N=4096, C=768, H=12, D=64) with bilinearly
interpolated relative position bias, on 8 Trainium2 NeuronCores.

Sharding: core c handles batch b = c//4 and heads 3*(c%4) .. 3*(c%4)+2
(tensor-parallel over heads, data-parallel over batch). Each core outputs, per
head, the unnormalized attention output projected through Wo_h (c-major), plus
the softmax denominators; the host divides, sums the partials, and adds bo.

Device algorithm per core (fp16 matmul operands, fp32 accumulation):
  qbT[h]  = (Wq_h^T @ q^T) * scale + bq          [64, 1024]   (d-major)
  kbT[h]  = Wk_h^T @ kv^T + bk                   [64, 4096]
  vb[n]   = kv @ Wv_h + bv                       [4096, 64]   (n-major)
  S^T     = [kbT; Wn]^T-contraction [qbT; B1T]   K=96 fuses the interpolated
            bias: bias[h,q,n] = sum_j B1[h,q,j] * Wn[j,n]
  E^T     = exp(S^T)            (no max-subtraction; logits are O(1))
  O^T[h]  = [vb_h | 1]^T @ E^T                   [65, 1024]  row 64 = denom
  G^T[h]  = Wo_h^T-contraction O^T[h]            [768, 1024]  (c-major, unnorm)

Scheduling: kv is DMA'd in 8 column chunks ordered so the k/v projection of
chunk 0 (B0) can start ~2.5us in; a short warm-up matmul burst covers the
initial DMA latency and holds the HAM clock gate at 8/8.  The k/v projections
of chunks 2-7 are split into 4 sub-parts each and emitted one sub-part per
attention chunk of head 0 so the exp pipeline is never starved by a long
projection burst.  Wo for heads 0/1 is spread one (cs,qh) slice per attention
chunk of heads 1/2; head 2's Wo forms the tail with copies alternating between
the vector and scalar engines.
"""

import numpy as np

B, Q, N, C = 2, 1024, 4096, 768
H, D, REL = 12, 64, 32
SCALE = 1.0 / np.sqrt(D)
HPC = 3            # heads per core
N_CORES = 8
NCH = 8            # 512-wide n-chunks

_COMPILED = None   # cached nc across kernel() calls


def _lin_coords(n_out, n_in):
    pos = np.arange(n_out, dtype=np.float32) * np.float32((n_in - 1) / (n_out - 1))
    lo = np.clip(np.floor(pos).astype(np.int32), 0, n_in - 1)
    hi = np.clip(lo + 1, 0, n_in - 1)
    w = (pos - lo.astype(np.float32)).astype(np.float32)
    return lo, hi, w


def _host_bias_parts(rel_pos_bias):
    """B1: [H, Q, 32] q-interpolated bias; Wn: [32, N] n-interp weights."""
    lq, hq, wq = _lin_coords(Q, REL)
    ln, hn, wn = _lin_coords(N, REL)
    b1 = (rel_pos_bias[:, lq, :] * (1.0 - wq)[None, :, None]
          + rel_pos_bias[:, hq, :] * wq[None, :, None]).astype(np.float32)
    w_n = np.zeros((REL, N), np.float32)
    np.add.at(w_n, (ln, np.arange(N)), (1.0 - wn))
    np.add.at(w_n, (hn, np.arange(N)), wn)
    return b1, w_n


def _build():
    import concourse.tile as tile
    from concourse import bacc, mybir
    import concourse.bass as bass

    F32 = mybir.dt.float32
    F16 = mybir.dt.float16
    KT = 6  # C // 128 contraction tiles

    nc = bacc.Bacc("TRN2", target_bir_lowering=False, debug=False,
                   enable_asserts=False, num_devices=N_CORES)

    # DRAM layouts are partition-major / chunk-contiguous so every transfer
    # is one dma_start with large contiguous runs — the DGE descriptor
    # generator on the Sync queue is the startup bottleneck, not HBM BW.
    qT = nc.dram_tensor("qT", [128, KT, Q], F16, kind="ExternalInput")
    kvd = nc.dram_tensor("kvd", [NCH, 128, KT, 512], F16, kind="ExternalInput")
    wq = nc.dram_tensor("wq", [128, KT, 192], F16, kind="ExternalInput")  # pre-scaled
    wk = nc.dram_tensor("wk", [128, KT, 192], F16, kind="ExternalInput")
    wv = nc.dram_tensor("wv", [128, KT, 192], F16, kind="ExternalInput")
    wo = nc.dram_tensor("wo", [D, HPC, C], F16, kind="ExternalInput")
    bqs = nc.dram_tensor("bqs", [D, HPC], F32, kind="ExternalInput")   # pre-scaled
    bks = nc.dram_tensor("bks", [D, HPC], F32, kind="ExternalInput")
    bvb = nc.dram_tensor("bvb", [128, 192], F32, kind="ExternalInput")  # bcast bv
    b1t = nc.dram_tensor("b1t", [HPC, REL, Q], F16, kind="ExternalInput")
    w_n = nc.dram_tensor("w_n", [REL, N], F16, kind="ExternalInput")
    out_p = nc.dram_tensor("out_p", [HPC, 6, 128, 2, 512], F16,
                           kind="ExternalOutput")
    den = nc.dram_tensor("den", [HPC, Q], F16, kind="ExternalOutput")

    EXP = mybir.ActivationFunctionType.Exp
    ADD = mybir.AluOpType.add

    with tile.TileContext(nc) as tc:
        with (
            tc.tile_pool(name="wpool", bufs=1) as wpool,
            tc.tile_pool(name="persist", bufs=1) as pers,
            tc.tile_pool(name="psS", bufs=2, space="PSUM") as psSp,
            tc.tile_pool(name="pexp", bufs=4) as pexp,
            tc.tile_pool(name="tailp", bufs=2) as tailp,
        ):
            # ---- PE warm-up: keep the array busy until the first DMAs land
            # so the HAM clock gate reaches 8/8; also preloads the Exp table.
            scr = wpool.tile([128, 256], F16, name="scr")
            nc.vector.memset(scr, 0.0)
            dum = wpool.tile([1, 16], F32, name="dum")
            nc.vector.memset(dum, 0.0)
            dux = wpool.tile([1, 16], F16, name="dux")
            nc.scalar.activation(out=dux, in_=dum, func=EXP)
            with tc.tile_pool(name="pswrm", bufs=1, space="PSUM") as pswrm:
                psw = pswrm.tile([128, 256], F32, name="psw")
                for _ in range(40):
                    nc.tensor.matmul(psw, scr[:, 0:128], scr,
                                     start=True, stop=True)

            # ---- inputs: one dma_start per tensor (or per kv chunk), in
            # priority order.  The DGE issues them serially, so program
            # order IS the priority order. ----
            kv_sb = wpool.tile([128, NCH, KT, 512], F16, name="kv_sb")
            wk_sb = wpool.tile([128, KT, 192], F16, name="wk_sb")
            wv_sb = wpool.tile([128, KT, 192], F16, name="wv_sb")
            wq_sb = wpool.tile([128, KT, 192], F16, name="wq_sb")
            qT_sb = wpool.tile([128, KT, Q], F16, name="qT_sb")
            wo_sb = wpool.tile([D, HPC, C], F16, name="wo_sb")
            bks_sb = wpool.tile([D, HPC], F32, name="bks_sb")
            bvb_sb = wpool.tile([128, 192], F32, name="bvb_sb")
            bqs_sb = wpool.tile([D, HPC], F32, name="bqs_sb")
            wn_sb = wpool.tile([128, N], F16, name="wn_sb")  # rows 64:96 used
            # Two parallel DGE queues: phase A's inputs go through the
            # scalar engine's DGE (idle at startup), B0's through sync.
            nc.scalar.dma_start(out=wq_sb, in_=wq[:, :, :])
            nc.scalar.dma_start(out=bqs_sb, in_=bqs[:, :])
            nc.scalar.dma_start(out=qT_sb, in_=qT[:, :, :])
            nc.sync.dma_start(out=kv_sb[:, 0, :, :], in_=kvd[0, :, :, :])
            nc.sync.dma_start(out=wk_sb, in_=wk[:, :, :])
            nc.sync.dma_start(out=wv_sb, in_=wv[:, :, :])
            nc.sync.dma_start(out=bks_sb, in_=bks[:, :])
            nc.sync.dma_start(out=bvb_sb, in_=bvb[:, :])
            nc.sync.dma_start(out=wn_sb[64:96, :], in_=w_n[:, :])
            nc.sync.dma_start(out=kv_sb[:, 1, :, :], in_=kvd[1, :, :, :])

            # ---- persistent per-head / per-chunk tiles ----
            qTp = [pers.tile([96, Q], F16, name=f"qTp{h}", tag=f"qTp{h}")
                   for h in range(HPC)]
            for h in range(HPC):
                nc.scalar.dma_start(out=qTp[h][64:96, :], in_=b1t[h, :, :])
            kbTp = [[pers.tile([96, 512], F16, name=f"kbT{h}_{ch}",
                               tag=f"kbT{h}_{ch}")
                     for ch in range(NCH)] for h in range(HPC)]
            vb = [pers.tile([128, 195], F16, name=f"vb{s}", tag=f"vb{s}")
                  for s in range(N // 128)]

            # ---- phase B sub-parts: k/v projections for one 512-wide n-chunk
            # split into 4 pieces so they interleave with attention chunks ----
            def emit_b_sub(ch, part, psB):
                ksl = slice(512 * ch, 512 * ch + 512)
                if part == 0:
                    pskA = psB.tile([128, 512], F32, name="pskA", tag="psb")
                    for t in range(KT):
                        nc.tensor.matmul(pskA, wk_sb[:, t, 0:128],
                                         kv_sb[:, ch, t, :],
                                         start=(t == 0), stop=(t == KT - 1))
                    nc.vector.tensor_scalar_add(kbTp[0][ch][0:64, :],
                                                pskA[0:64, :], bks_sb[:, 0:1])
                    nc.vector.tensor_scalar_add(kbTp[1][ch][0:64, :],
                                                pskA[64:128, :], bks_sb[:, 1:2])
                elif part == 1:
                    pskB = psB.tile([64, 512], F32, name="pskB", tag="psb")
                    for t in range(KT):
                        nc.tensor.matmul(pskB, wk_sb[:, t, 128:192],
                                         kv_sb[:, ch, t, :],
                                         start=(t == 0), stop=(t == KT - 1))
                    nc.vector.tensor_scalar_add(kbTp[2][ch][0:64, :],
                                                pskB[0:64, :], bks_sb[:, 2:3])
                    for h in range(HPC):
                        nc.vector.tensor_copy(out=kbTp[h][ch][64:96, :],
                                              in_=wn_sb[64:96, ksl])
                else:
                    for s in (0, 1) if part == 2 else (2, 3):
                        n128 = 4 * ch + s
                        psv = psB.tile([128, 192], F32, name="psv", tag="psb")
                        for t in range(KT):
                            nc.tensor.matmul(
                                psv, kv_sb[:, ch, t, 128 * s:128 * s + 128],
                                wv_sb[:, t, :],
                                start=(t == 0), stop=(t == KT - 1))
                        vt = vb[n128]
                        vt3 = vt[:, 0:195].rearrange("p (h e) -> p h e", e=65)
                        nc.vector.tensor_tensor(
                            out=vt3[:, :, 0:64],
                            in0=psv.rearrange("p (h d) -> p h d", d=64),
                            in1=bvb_sb.rearrange("p (h d) -> p h d", d=64),
                            op=ADD)
                        nc.vector.memset(vt3[:, :, 64:65], 1.0)

            def emit_b_chunk(ch, psB):
                for part in range(4):
                    emit_b_sub(ch, part, psB)

            # ---- attention S matmuls: one chunk of one head ----
            def emit_s(h, ci):
                c512, s = ci // 4, ci % 4
                ssl = slice(128 * s, 128 * s + 128)
                psS = psSp.tile([128, Q], F32, name="psS", tag="psS")
                nc.tensor.matmul(psS[:, 0:512], kbTp[h][c512][:, ssl],
                                 qTp[h][:, 0:512], start=True, stop=True)
                nc.tensor.matmul(psS[:, 512:1024], kbTp[h][c512][:, ssl],
                                 qTp[h][:, 512:1024], start=True, stop=True)
                return psS

            # ---- Wo: one (cs, qh) slice => 1 matmul + 1 copy + 1 dma ----
            def emit_wo_slice(h, onT, idx, psF, scalar_copy=False):
                cs, qh = idx // 2, idx % 2
                csl = slice(128 * cs, 128 * cs + 128)
                qsl = slice(512 * qh, 512 * qh + 512)
                psf = psF.tile([128, 512], F32, name="psf", tag="psf")
                nc.tensor.matmul(psf, wo_sb[:, h, csl], onT[0:64, qsl],
                                 start=True, stop=True)
                osb = tailp.tile([128, 512], F16, name="osb", tag="osb", bufs=4)
                if scalar_copy:
                    nc.scalar.copy(out=osb, in_=psf)
                else:
                    nc.vector.tensor_copy(out=osb, in_=psf)
                nc.sync.dma_start(out=out_p[h, cs, :, qh, :], in_=osb)

            import contextlib
            psB_ctx = contextlib.ExitStack()
            psB_pool = psB_ctx.enter_context(
                tc.tile_pool(name="psB", bufs=2, space="PSUM"))
            emit_b_chunk(0, psB_pool)

            # ---- phase A: q projection ----
            with tc.tile_pool(name="psA0", bufs=1, space="PSUM") as psA0:
                for qc in range(2):
                    psqA = psA0.tile([128, 512], F32, name="psqA", tag="psqA")
                    psqB = psA0.tile([64, 512], F32, name="psqB", tag="psqB")
                    for t in range(KT):
                        nc.tensor.matmul(psqA, wq_sb[:, t, 0:128],
                                         qT_sb[:, t, 512 * qc:512 * qc + 512],
                                         start=(t == 0), stop=(t == KT - 1))
                        nc.tensor.matmul(psqB, wq_sb[:, t, 128:192],
                                         qT_sb[:, t, 512 * qc:512 * qc + 512],
                                         start=(t == 0), stop=(t == KT - 1))
                    sl = slice(512 * qc, 512 * qc + 512)
                    nc.vector.tensor_scalar_add(qTp[0][0:64, sl], psqA[0:64, :],
                                                bqs_sb[:, 0:1])
                    nc.vector.tensor_scalar_add(qTp[1][0:64, sl], psqA[64:128, :],
                                                bqs_sb[:, 1:2])
                    nc.vector.tensor_scalar_add(qTp[2][0:64, sl], psqB[0:64, :],
                                                bqs_sb[:, 2:3])

            emit_b_chunk(1, psB_pool)
            for ch in range(2, NCH):
                nc.sync.dma_start(out=kv_sb[:, ch, :, :], in_=kvd[ch, :, :, :])
            nc.sync.dma_start(out=wo_sb, in_=wo[:, :, :])

            NCI = N // 128

            def emit_head(h, psB, wo_work):
                """wo_work: None or (h_prev, onT_prev, psF)."""
                import contextlib
                with contextlib.ExitStack() as pstk:
                    psO = pstk.enter_context(
                        tc.tile_pool(name=f"psO{h}", bufs=1, space="PSUM"))
                    po = psO.tile([65, Q], F32, name=f"po{h}", tag="po")
                    psS_cur = emit_s(h, 0)
                    wo_idx = 0
                    for ci in range(NCI):
                        # run-ahead: next chunk's S matmuls first so exp
                        # never waits on them
                        if ci + 1 < NCI:
                            psS_nxt = emit_s(h, ci + 1)
                        # one sub-part of a pending k/v projection chunk
                        if psB is not None and ci < 24:
                            emit_b_sub(2 + ci // 4, ci % 4, psB)
                        # one Wo slice of the previous head
                        if wo_work is not None and 2 <= ci and wo_idx < 12:
                            hp, onTp, psF = wo_work
                            emit_wo_slice(hp, onTp, wo_idx, psF)
                            wo_idx += 1
                        px = pexp.tile([128, Q], F16, name="px", tag="px")
                        nc.scalar.activation(out=px, in_=psS_cur, func=EXP)
                        st = (ci == 0)
                        sp = (ci == NCI - 1)
                        nc.tensor.matmul(po[:, 0:512],
                                         vb[ci][:, 65 * h:65 * h + 65],
                                         px[:, 0:512], start=st, stop=sp)
                        nc.tensor.matmul(po[:, 512:1024],
                                         vb[ci][:, 65 * h:65 * h + 65],
                                         px[:, 512:1024], start=st, stop=sp)
                        if ci + 1 < NCI:
                            psS_cur = psS_nxt
                    # single copy grabs the 64 output dims AND the denom row
                    onT = tailp.tile([65, Q], F16, name=f"onT{h}", tag="onT")
                    nc.vector.tensor_copy(out=onT, in_=po)
                    nc.sync.dma_start(out=den[h, :], in_=onT[64:65, :])
                return onT

            onT0 = emit_head(0, psB_pool, None)
            psB_ctx.close()
            psF_ctx = contextlib.ExitStack()
            psF_pool = psF_ctx.enter_context(
                tc.tile_pool(name="psF", bufs=2, space="PSUM"))
            onT1 = emit_head(1, None, (0, onT0, psF_pool))
            onT2 = emit_head(2, None, (1, onT1, psF_pool))
            # tail: head 2's Wo, copies alternating vector/scalar engines
            for idx in range(12):
                emit_wo_slice(2, onT2, idx, psF_pool, scalar_copy=(idx % 2 == 1))
            psF_ctx.close()
    nc.compile()
    return nc


def _get_compiled():
    global _COMPILED
    if _COMPILED is None:
        _COMPILED = _build()
    return _COMPILED


def _make_in_maps(query, key_value, Wq, bq, Wk, bk, Wv, bv, Wo, rel_pos_bias):
    b1, w_n = _host_bias_parts(rel_pos_bias)
    scale = np.float32(SCALE)
    f16 = np.float16
    qTs = [np.ascontiguousarray(
        query[b].T.reshape(6, 128, Q).transpose(1, 0, 2)).astype(f16)
        for b in range(B)]
    kvds = [np.ascontiguousarray(
        key_value[b].T.reshape(6, 128, NCH, 512).transpose(2, 1, 0, 3)
        ).astype(f16) for b in range(B)]
    def wlay(w):
        return np.ascontiguousarray(
            w.reshape(6, 128, 192).transpose(1, 0, 2)).astype(f16)
    w_n16 = w_n.astype(f16)
    in_maps = []
    for c in range(N_CORES):
        b = c // (N_CORES // B)
        h0 = (c % (N_CORES // B)) * HPC
        cols = slice(D * h0, D * h0 + D * HPC)
        in_maps.append({
            "qT": qTs[b],
            "kvd": kvds[b],
            "wq": wlay(Wq[:, cols] * scale),
            "wk": wlay(Wk[:, cols]),
            "wv": wlay(Wv[:, cols]),
            "wo": np.ascontiguousarray(
                Wo[cols, :].reshape(HPC, D, C).transpose(1, 0, 2)).astype(f16),
            "bqs": np.ascontiguousarray((bq[cols] * scale).reshape(HPC, D).T),
            "bks": np.ascontiguousarray(bk[cols].reshape(HPC, D).T),
            "bvb": np.ascontiguousarray(
                np.broadcast_to(bv[cols][None, :], (128, D * HPC))),
            "b1t": np.ascontiguousarray(
                b1[h0:h0 + HPC].transpose(0, 2, 1)).astype(f16),
            "w_n": w_n16,
        })
    return in_maps


def kernel(query, key_value, Wq, bq, Wk, bk, Wv, bv, Wo, bo, rel_pos_bias):
    from concourse import bass_utils

    query = np.asarray(query, np.float32)
    key_value = np.asarray(key_value, np.float32)
    Wq = np.asarray(Wq, np.float32); bq = np.asarray(bq, np.float32)
    Wk = np.asarray(Wk, np.float32); bk = np.asarray(bk, np.float32)
    Wv = np.asarray(Wv, np.float32); bv = np.asarray(bv, np.float32)
    Wo = np.asarray(Wo, np.float32); bo = np.asarray(bo, np.float32)
    rel_pos_bias = np.asarray(rel_pos_bias, np.float32)

    in_maps = _make_in_maps(query, key_value, Wq, bq, Wk, bk, Wv, bv, Wo,
                            rel_pos_bias)
    nc = _get_compiled()
    res = bass_utils.run_bass_kernel_spmd(nc, in_maps,
                                          core_ids=list(range(N_CORES)))
    out = np.zeros((B, Q, C), np.float32)
    for c in range(N_CORES):
        b = c // (N_CORES // B)
        f = res.results[c]["out_p"].reshape(HPC, C, Q).astype(np.float32)
        d = res.results[c]["den"].astype(np.float32)     # [HPC, Q]
        out[b] += (f / d[:, None, :]).sum(axis=0).T
    out += bo[None, None, :]
    return out
